# revision 18
# baseline (speedup 1.0000x reference)
"""Trainium2 Bass kernel for AngularMinPooling.

out[v, r] = inputs[v, r, argmin_j ||inputs[v, j, :]||_2]
Input (500000, 8, 64) f32 -> Output (500000, 8) f32.
Vertices are sharded across 8 NeuronCores; no cross-core communication.

Per 128x16-vertex tile: ACT saves the first 8 feature columns (all the
gather can touch) then squares the tile in place; DVE does the
segmented sum-reduce to squared norms plus the min/max reduces of the
first-min argmin encoding (tie-broken toward the lowest rotation index
to match argmin); GPSIMD does the small compare/select elementwise ops
and the one-hot gather multiply. Built as a Bacc graph so sync waits
are legalized (TRN2 allows 1 wait per instruction). Output is staged in
SBUF for the whole shard and written once, partition-major; the host
undoes the permutation.
"""

import os
import sys

import numpy as np

for _p in ("/opt/trn_rl_repo",):
    if os.path.isdir(_p) and _p not in sys.path:
        sys.path.insert(0, _p)

import concourse.bacc as bacc
import concourse.bass as bass
import concourse.tile as tile
from concourse import mybir
from concourse.bass_utils import run_bass_kernel_spmd


def _ensure_ntff_hook():
    """Install the axon NTFF profile hook if the image's antenv lacks it.

    Mirrors trn_boot.py section 6; makes run(..., trace=True) return
    exec_time_ns + perfetto trace instead of silently skipping.
    """
    import types

    try:
        from antenv.axon_hooks import get_axon_ntff_profile_hook  # noqa: F401

        return
    except ImportError:
        pass
    try:
        import antenv
        from trn_agent_boot.trn_boot import _ntff_profile_via_ctypes

        mod = types.ModuleType("antenv.axon_hooks")
        _state = {"hook": None}
        mod.set_axon_ntff_profile_hook = lambda h: _state.__setitem__("hook", h)
        mod.get_axon_ntff_profile_hook = lambda: _state["hook"]
        sys.modules["antenv.axon_hooks"] = mod
        antenv.axon_hooks = mod
        so_path = "/opt/axon/libaxon_pjrt.so"
        if os.path.exists(so_path):
            mod.set_axon_ntff_profile_hook(_ntff_profile_via_ctypes(so_path))
    except Exception:
        pass


_ensure_ntff_hook()

N_VERTICES = 500_000
R = 8
F = 64
N_CORES = 8
N_SHARD = N_VERTICES // N_CORES  # 62500 vertices per core
P = 128  # SBUF partitions
VPP = 16  # vertices per partition per full tile
TILE_V = P * VPP  # 2048 vertices per full tile

# Tile plan per core: full tiles of 128x16, then a 128x8 tile, then the
# 36-vertex remainder. (62500 = 30*2048 + 1024 + 36)
N_FULL = N_SHARD // TILE_V  # 30
_rem = N_SHARD - N_FULL * TILE_V  # 1060
VPP_MID = _rem // P  # 8
TAIL = _rem - VPP_MID * P  # 36
N_SLOTS = N_FULL * VPP + VPP_MID  # 488 staged vertex slots per partition

_DT = mybir.dt.float32
_AX = mybir.AxisListType
_OP = mybir.AluOpType


def _build_nc():
    nc = bacc.Bacc(
        "TRN2",
        target_bir_lowering=False,
        debug=False,
        enable_asserts=False,
        num_devices=N_CORES,
    )
    x = nc.dram_tensor("inputs", [N_SHARD, R, F], _DT, kind="ExternalInput")
    # Partition-major staged output: raw[p, s, r]; host maps slots back to
    # vertex order per tile.
    raw = nc.dram_tensor("raw", [P, N_SLOTS, R], _DT, kind="ExternalOutput")
    traw = nc.dram_tensor("traw", [TAIL, R], _DT, kind="ExternalOutput")
    xa = x.ap()

    with tile.TileContext(nc) as tc:
        with (
            tc.tile_pool(name="xin", bufs=3) as xin_pool,
            tc.tile_pool(name="big", bufs=2) as big_pool,
            tc.tile_pool(name="work", bufs=4) as work_pool,
            tc.tile_pool(name="stage", bufs=1) as stage_pool,
            tc.tile_pool(name="const", bufs=1) as const_pool,
        ):
            # rev[j] = R - j: multiplying the min-mask by this and taking the
            # max yields the FIRST (lowest-index) min, matching argmin.
            rev = const_pool.tile([P, R], _DT)
            for j in range(R):
                nc.vector.memset(rev[:, j : j + 1], float(R - j))

            stage = stage_pool.tile([P, N_SLOTS, R], _DT)

            def do_tile(idx, v0, pc, vpp, ot_dst):
                xt = xin_pool.tile([P, VPP, R, F], _DT, tag="xt")
                src = xa[v0 : v0 + pc * vpp].rearrange("(p v) r f -> p v r f", p=pc)
                dma_eng = nc.sync if idx % 2 == 0 else nc.scalar
                dma_eng.dma_start(out=xt[:pc, :vpp], in_=src)

                # Save the gatherable columns, then square in place (ACT).
                xs8 = big_pool.tile([P, VPP, R, R], _DT, tag="xs8")
                nc.scalar.copy(xs8[:pc, :vpp], xt[:pc, :vpp, :, 0:R])
                nc.scalar.square(xt[:pc, :vpp], xt[:pc, :vpp])

                # GPSIMD pre-sums the squared halves so DVE only reduces half
                # the elements.
                h = big_pool.tile([P, VPP, R, F // 2], _DT, tag="h")
                nc.gpsimd.tensor_add(
                    h[:pc, :vpp],
                    xt[:pc, :vpp, :, 0 : F // 2],
                    xt[:pc, :vpp, :, F // 2 : F],
                )
                sq = work_pool.tile([P, VPP, R], _DT, tag="sq")
                nc.vector.tensor_reduce(
                    out=sq[:pc, :vpp], in_=h[:pc, :vpp], axis=_AX.X, op=_OP.add
                )
                m = work_pool.tile([P, VPP], _DT, tag="m")
                nc.vector.tensor_reduce(
                    out=m[:pc, :vpp], in_=sq[:pc, :vpp], axis=_AX.X, op=_OP.min
                )
                oh = work_pool.tile([P, VPP, R], _DT, tag="oh")
                nc.vector.tensor_tensor(
                    out=oh[:pc, :vpp],
                    in0=sq[:pc, :vpp],
                    in1=m[:pc, :vpp, None].broadcast_to([pc, vpp, R]),
                    op=_OP.is_le,
                )
                enc = work_pool.tile([P, VPP, R], _DT, tag="enc")
                nc.vector.tensor_tensor(
                    out=enc[:pc, :vpp],
                    in0=oh[:pc, :vpp],
                    in1=rev[:pc, None, :].broadcast_to([pc, vpp, R]),
                    op=_OP.mult,
                )
                mx = work_pool.tile([P, VPP], _DT, tag="mx")
                nc.vector.tensor_reduce(
                    out=mx[:pc, :vpp], in_=enc[:pc, :vpp], axis=_AX.X, op=_OP.max
                )
                sel = work_pool.tile([P, VPP, R], _DT, tag="sel")
                nc.vector.tensor_tensor(
                    out=sel[:pc, :vpp],
                    in0=enc[:pc, :vpp],
                    in1=mx[:pc, :vpp, None].broadcast_to([pc, vpp, R]),
                    op=_OP.is_equal,
                )
                # Gather via one-hot weighted sum over the first R feature
                # columns (argmin index is always < R).
                g = big_pool.tile([P, VPP, R, R], _DT, tag="g")
                nc.vector.tensor_tensor(
                    out=g[:pc, :vpp],
                    in0=xs8[:pc, :vpp],
                    in1=sel[:pc, :vpp, None, :].broadcast_to([pc, vpp, R, R]),
                    op=_OP.mult,
                )
                nc.vector.tensor_reduce(
                    out=ot_dst, in_=g[:pc, :vpp], axis=_AX.X, op=_OP.add
                )

            for t in range(N_FULL):
                do_tile(t, t * TILE_V, P, VPP, stage[:, t * VPP : (t + 1) * VPP])
            if VPP_MID:
                do_tile(
                    N_FULL,
                    N_FULL * TILE_V,
                    P,
                    VPP_MID,
                    stage[:, N_FULL * VPP : N_FULL * VPP + VPP_MID],
                )
            nc.sync.dma_start(out=raw.ap(), in_=stage[:])

            if TAIL:
                ot_tail = work_pool.tile([P, VPP, R], _DT, tag="ot_tail")
                do_tile(
                    N_FULL + 1,
                    N_FULL * TILE_V + VPP_MID * P,
                    TAIL,
                    1,
                    ot_tail[:TAIL, :1],
                )
                nc.sync.dma_start(out=traw.ap(), in_=ot_tail[:TAIL, :1])
    nc.finalize()
    return nc


_NC_CACHE = None


def _get_nc():
    global _NC_CACHE
    if _NC_CACHE is None:
        _NC_CACHE = _build_nc()
    return _NC_CACHE


def _decode_raw(raw_arr: np.ndarray, traw_arr: np.ndarray) -> np.ndarray:
    """Map staged [P, N_SLOTS, R] output back to vertex order."""
    parts = []
    slot = 0
    for vpp in [VPP] * N_FULL + ([VPP_MID] if VPP_MID else []):
        blk = raw_arr[:, slot : slot + vpp, :]  # [P, vpp, R]
        parts.append(blk.reshape(P * vpp, R))
        slot += vpp
    if TAIL:
        parts.append(traw_arr)
    return np.concatenate(parts, axis=0)


def run(inputs: np.ndarray, **spmd_kwargs):
    inputs = np.ascontiguousarray(np.asarray(inputs, dtype=np.float32))
    assert inputs.shape == (N_VERTICES, R, F), inputs.shape
    shards = np.split(inputs, N_CORES, axis=0)
    in_maps = [{"inputs": np.ascontiguousarray(s)} for s in shards]
    res = run_bass_kernel_spmd(
        _get_nc(), in_maps, core_ids=list(range(N_CORES)), **spmd_kwargs
    )
    out = np.concatenate(
        [_decode_raw(r["raw"], r["traw"]) for r in res.results], axis=0
    )
    return out, res


def kernel(inputs: np.ndarray) -> np.ndarray:
    out, _ = run(inputs)
    return out
